# revision 30
# baseline (speedup 1.0000x reference)
"""Masked-loss kernel for nn_MLoss_9715216024200 on 8 Trainium2 NeuronCores.

loss = sum(where(y[...,0]>0.5, (y-x)^2 - a*x^2, 0)) + a*sum(x[...,0]^2)
with x,y f32 (256, 10647, 5); output is a f32 scalar.

Sharding: flatten to cells, pad with 256 zero cells (neutral: y0=0 ->
mask 0, x=0 -> no bg term), split across (8 cores, 128 partitions,
2662 cells), and ship the shards as bf16 in FEATURE-PLANE layout: each
core's x (and y) is [128, 5*2662] with the 5 features stored as packed
per-feature planes.  bf16 halves the HBM stream to ~19us at the 360GB/s
DMA roofline (the loss tolerates it: rel err ~1e-4 << 2e-2), and the
plane layout makes the mask a PACKED [128, w] tensor -- no per-feature
replication, so the mask costs 0.28ns/cell (DVE tensor_scalar 4x mode)
instead of 1.4ns/elem on Pool.

Per-core math uses mask idempotence (m in {0,1} => m^2 = m):

  sum(m*(d^2 - a*x^2)) = sum(m*y^2) - 2*sum((m*x) o y) + (1-a)*sum((m*x)^2)

The big sums are Gram-matrix diagonals on the otherwise idle PE: per
128-cell block and feature plane, matmul(psA += mx_f^T y_f),
matmul(psB += mx_f^T x_f), matmul(psC += x0^T x0), and for 'p'-mode
tiles matmul(psD += m^T y2_f) (53ns per matmul; the host takes trace()
of the four staged 128x128 Grams).  PE matmuls are emitted PE_LAG tiles
behind the stream so PE ramps to full clock once and stays there.
sum(m*y^2) is per-tile routed by MYSQ: 'a' = my=m*y (part of one
10-plane masked multiply) + ACT Square-accum; 'p' = y2=y*y (DVE/ACT/
Pool by knob) + PE D-Gram.  The tail tile skips PE (DVE ttr cross +
my^2, ACT mx^2 + bg) so the Gram psums close at tile N-2 and their
export overlaps the drain.  Host combines everything in f64.

Per-elem rates: DVE tt/ts packed bf16 0.52/0.28, DVE ttr 1.05, ACT
square 0.83 (+0.6us/instr), Pool tt 1.98, PE 0.41 per Gram term.
Engines land at ~10-15us each, under the ~19.3us DMA stream.
"""
import sys

for _p in ('/opt/trn_rl_repo',):
    if _p in sys.path:
        sys.path.remove(_p)
    sys.path.insert(0, _p)

import os as _os
import numpy as np

B, C, F = 256, 10647, 5
THRESH = 0.5
ALPHA = 0.1
N_CORES = 8
P = 128
CELLS = B * C                      # 2,725,632
CPP = 2662                         # cells/partition; 8*128*2662 = 2,725,888
PAD_CELLS = N_CORES * P * CPP - CELLS   # 256
FD = CPP * F                       # 13310 elems per partition per core

_ts = _os.environ.get('TILE_SIZES', '')
TILE_SIZES = ([int(v) for v in _ts.split(',')] if _ts
              else [102] + [256] * 10)
assert sum(TILE_SIZES) == CPP
N_TILES = len(TILE_SIZES)
# per-tile my^2 route: 'a' = my + ACT square-accum, 'p' = y2 + PE D-Gram,
# 'v' = off-PE tile (DVE ttr cross + my^2, ACT mx^2 + bg) -- put 'v'
# tiles FIRST in the stream so their slow chains run early
_mm = _os.environ.get('MYSQ', 'v,a,p,a,p,a,p,a,p,p,p')
MYSQ = _mm.split(',')
assert len(MYSQ) == N_TILES
# engine for y2 on 'p' tiles: tiles listed in Y2_ACT use ACT, Y2_POOL use
# Pool, the rest DVE
_ya = _os.environ.get('Y2_ACT', '8')
Y2_ACT = set(int(v) for v in _ya.split(',') if v != '')
_yp = _os.environ.get('Y2_POOL', '2,4,6')
Y2_POOL = set(int(v) for v in _yp.split(',') if v != '')
# 'a' tiles whose my-planes run on Pool (separate from the mx multiply)
_mg = _os.environ.get('MY_POOL', '1')
MY_POOL = set(int(v) for v in _mg.split(',') if v != '')
# flush tile t's PE matmuls after tile t+PE_LAG's products are emitted
PE_LAG = int(_os.environ.get('PE_LAG', '2'))
# defer 'v' tiles' compute emission by this many tiles
V_DEFER = int(_os.environ.get('V_DEFER', '2'))
BUFS = [int(v) for v in _os.environ.get('BUFS', '6,6,5,3').split(',')]

_compiled = None


def _build():
    from contextlib import ExitStack
    import concourse.tile as tile
    from concourse import bacc, mybir

    sqa = float(np.sqrt(ALPHA))

    nc = bacc.Bacc("TRN2", target_bir_lowering=False, debug=False,
                   enable_asserts=True, num_devices=N_CORES)
    bf16 = mybir.dt.bfloat16
    f32 = mybir.dt.float32
    x_d = nc.dram_tensor("x", [P, FD], bf16, kind="ExternalInput").ap()
    y_d = nc.dram_tensor("y", [P, FD], bf16, kind="ExternalInput").ap()
    o_d = nc.dram_tensor("o", [P, 4 * N_TILES], f32, kind="ExternalOutput").ap()
    g_d = nc.dram_tensor("g", [P, 512], bf16, kind="ExternalOutput").ap()

    Sq = mybir.ActivationFunctionType.Square
    Alu = mybir.AluOpType

    pe_tiles = [t for t in range(N_TILES) if MYSQ[t] != 'v']
    last_pe = pe_tiles[-1]
    last_p = max((t for t in pe_tiles if MYSQ[t] == 'p'), default=-1)
    # psum col ranges: A=cross(mx,y)  B=(mx)^2  C=bg x0^2  D=m o y^2
    first_pe = [True, True, True, True]

    x3 = x_d.rearrange("p (f c) -> p f c", f=F)
    y3 = y_d.rearrange("p (f c) -> p f c", f=F)

    with tile.TileContext(nc) as tc, ExitStack() as ctx:
        xyp = ctx.enter_context(tc.tile_pool(name="xy", bufs=BUFS[0]))
        mp_ = ctx.enter_context(tc.tile_pool(name="m", bufs=BUFS[1]))
        wp = ctx.enter_context(tc.tile_pool(name="w", bufs=BUFS[2]))
        sp = ctx.enter_context(tc.tile_pool(name="s", bufs=BUFS[3]))
        ap_ = ctx.enter_context(tc.tile_pool(name="acc", bufs=1))
        pp = ctx.enter_context(tc.psum_pool(name="ps", bufs=1))

        acc = ap_.tile([P, 4 * N_TILES], f32)
        nc.vector.memset(acc[:], 0.0)
        gst = ap_.tile([P, 512], bf16)
        # one PSUM bank per Gram: hardware tracks accumulation groups
        # per bank, so interleaved A/B/C/D accumulation needs 4 banks
        psA = pp.tile([P, 512], f32)
        psB = pp.tile([P, 512], f32)
        psC = pp.tile([P, 512], f32)
        psD = pp.tile([P, 512], f32)

        def emit_pe(t, w, xy, m, mxv, y2v):
            # per 128-cell block: A/B per plane; C plane-0; D (if y2v) per
            # plane against the shared mask stationary
            last_ab = (t == last_pe)
            nb = (w + 127) // 128
            for j in range(nb):
                lo = j * 128
                wb = min(128, w - lo)
                lab = last_ab and (j == nb - 1)
                for f in range(F):
                    mxf = mxv[:, f * w + lo: f * w + lo + wb]
                    yf = xy[:, (F + f) * w + lo: (F + f) * w + lo + wb]
                    xf = xy[:, f * w + lo: f * w + lo + wb]
                    nc.tensor.matmul(psA[0:wb, 0:wb], mxf, yf,
                                     start=first_pe[0],
                                     stop=lab and f == F - 1,
                                     skip_group_check=True)
                    first_pe[0] = False
                    nc.tensor.matmul(psB[0:wb, 0:wb], mxf, xf,
                                     start=first_pe[1],
                                     stop=lab and f == F - 1,
                                     skip_group_check=True)
                    first_pe[1] = False
                    if y2v is not None:
                        y2f = y2v[:, f * w + lo: f * w + lo + wb]
                        nc.tensor.matmul(psD[0:wb, 0:wb],
                                         m[:, lo:lo + wb], y2f,
                                         start=first_pe[3],
                                         stop=(t == last_p and j == nb - 1
                                               and f == F - 1),
                                         skip_group_check=True)
                        first_pe[3] = False
                x0b = xy[:, lo:lo + wb]
                nc.tensor.matmul(psC[0:wb, 0:wb], x0b, x0b,
                                 start=first_pe[2], stop=lab,
                                 skip_group_check=True)
                first_pe[2] = False
            if last_ab:
                # stage Grams to SBUF + export (overlaps the tail)
                nc.vector.tensor_copy(gst[:, 0:128], psA[:, 0:128])
                nc.scalar.copy(gst[:, 128:256], psB[:, 0:128])
                nc.vector.tensor_copy(gst[:, 256:384], psC[:, 0:128])
                nc.scalar.copy(gst[:, 384:512], psD[:, 0:128])
                nc.sync.dma_start(g_d, gst[:])

        pe_pending = []
        v_pending = []

        def emit_epilogue(t, w, xy, sl):
            while pe_pending:
                emit_pe(*pe_pending.pop(0))
            ypl = xy[:, F * w:2 * F * w]
            m = mp_.tile([P, w], bf16, tag="m")
            nc.vector.tensor_scalar(m[:], xy[:, F * w:F * w + w],
                                    THRESH, None, op0=Alu.is_gt)
            y2t = wp.tile([P, F * w], bf16, tag="y2")
            nc.vector.tensor_tensor(y2t[:], ypl, ypl, op=Alu.mult)
            mxt = wp.tile([P, F * w], bf16, tag="mx")
            nb = (w + 127) // 128
            for f in range(F):
                nc.sync.dma_start(
                    xy[:, f * w:(f + 1) * w].unsqueeze(1),
                    x3[:, f:f + 1, sl])
                nc.vector.tensor_tensor(mxt[:, f * w:(f + 1) * w],
                                        xy[:, f * w:(f + 1) * w], m[:],
                                        op=Alu.mult)
                for j in range(nb):
                    lo = j * 128
                    wb = min(128, w - lo)
                    fin = (f == F - 1 and j == nb - 1)
                    mxf = mxt[:, f * w + lo: f * w + lo + wb]
                    yf = xy[:, (F + f) * w + lo: (F + f) * w + lo + wb]
                    xf = xy[:, f * w + lo: f * w + lo + wb]
                    y2f = y2t[:, f * w + lo: f * w + lo + wb]
                    nc.tensor.matmul(psA[0:wb, 0:wb], mxf, yf,
                                     start=first_pe[0], stop=fin,
                                     skip_group_check=True)
                    first_pe[0] = False
                    nc.tensor.matmul(psB[0:wb, 0:wb], mxf, xf,
                                     start=first_pe[1], stop=fin,
                                     skip_group_check=True)
                    first_pe[1] = False
                    nc.tensor.matmul(psD[0:wb, 0:wb],
                                     m[:, lo:lo + wb], y2f,
                                     start=first_pe[3], stop=fin,
                                     skip_group_check=True)
                    first_pe[3] = False
                    if f == 0:
                        nc.tensor.matmul(psC[0:wb, 0:wb], xf, xf,
                                         start=first_pe[2],
                                         stop=(j == nb - 1),
                                         skip_group_check=True)
                        first_pe[2] = False
            nc.vector.tensor_copy(gst[:, 0:128], psA[:, 0:128])
            nc.scalar.copy(gst[:, 128:256], psB[:, 0:128])
            nc.vector.tensor_copy(gst[:, 256:384], psC[:, 0:128])
            nc.scalar.copy(gst[:, 384:512], psD[:, 0:128])
            nc.sync.dma_start(g_d, gst[:])

        off = 0
        for t, w in enumerate(TILE_SIZES):
            epilogue = (t == N_TILES - 1 and MYSQ[t] == 'p')
            xy = xyp.tile([P, 2 * F * w], bf16, tag="xy")
            sl = slice(off, off + w)
            off += w
            # y first (mask depends on it), then x; plane-strided DMAs
            nc.sync.dma_start(
                xy[:, F * w:2 * F * w].rearrange("p (f c) -> p f c", f=F),
                y3[:, :, sl])
            if epilogue:
                # final tile: stream x per plane, pipelining mx + Gram
                # matmuls behind each plane so the psum close trails the
                # last DMA by well under a microsecond
                emit_epilogue(t, w, xy, sl)
                continue
            nc.sync.dma_start(
                xy[:, 0:F * w].rearrange("p (f c) -> p f c", f=F),
                x3[:, :, sl])
            xpl = xy[:, 0:F * w]          # x planes
            ypl = xy[:, F * w:2 * F * w]  # y planes

            # packed per-cell mask (4x DVE tensor_scalar)
            m = mp_.tile([P, w], bf16, tag="m")
            nc.vector.tensor_scalar(m[:], xy[:, F * w:F * w + w],
                                    THRESH, None, op0=Alu.is_gt)

            mode = MYSQ[t]
            y2v = None
            if mode == 'p':
                mxt = wp.tile([P, F * w], bf16, tag="mx")
                nc.vector.tensor_tensor(
                    mxt[:].rearrange("p (f c) -> p f c", f=F),
                    xpl.rearrange("p (f c) -> p f c", f=F),
                    m[:].unsqueeze(1).broadcast_to((P, F, w)), op=Alu.mult)
                mxv = mxt[:]
                y2t = wp.tile([P, F * w], bf16, tag="y2")
                y2_eng = (nc.scalar if t in Y2_ACT
                          else nc.gpsimd if t in Y2_POOL else nc.vector)
                if t in Y2_ACT:
                    nc.scalar.activation(y2t[:], ypl, Sq)
                else:
                    y2_eng.tensor_tensor(y2t[:], ypl, ypl, op=Alu.mult)
                y2v = y2t[:]
            elif mode == 'v':
                mxv = myv = None
            else:
                # masked multiply of all 10 planes (or x/y split DVE/Pool)
                mxy = wp.tile([P, 2 * F * w], bf16, tag="mx")
                if t in MY_POOL and mode == 'a':
                    nc.vector.tensor_tensor(
                        mxy[:, 0:F * w].rearrange("p (f c) -> p f c", f=F),
                        xpl.rearrange("p (f c) -> p f c", f=F),
                        m[:].unsqueeze(1).broadcast_to((P, F, w)),
                        op=Alu.mult)
                    nc.gpsimd.tensor_tensor(
                        mxy[:, F * w:].rearrange("p (f c) -> p f c", f=F),
                        ypl.rearrange("p (f c) -> p f c", f=F),
                        m[:].unsqueeze(1).broadcast_to((P, F, w)),
                        op=Alu.mult)
                else:
                    nc.vector.tensor_tensor(
                        mxy[:].rearrange("p (k c) -> p k c", k=2 * F),
                        xy[:].rearrange("p (k c) -> p k c", k=2 * F),
                        m[:].unsqueeze(1).broadcast_to((P, 2 * F, w)),
                        op=Alu.mult)
                mxv = mxy[:, 0:F * w]
                myv = mxy[:, F * w:2 * F * w]
                if mode == 'a':
                    sq = sp.tile([P, F * w], bf16, tag="sq")
                    nc.scalar.activation(sq[:], myv, Sq,
                                         accum_out=acc[:, t:t + 1])

            if mode != 'v':
                pe_pending.append((t, w, xy, m, mxv, y2v))
                while pe_pending and (pe_pending[0][0] + PE_LAG <= t
                                      or t == last_pe):
                    emit_pe(*pe_pending.pop(0))
            else:
                # off-PE tile: cross + my^2 on DVE ttr, mx^2 + bg on ACT.
                # Deferred V_DEFER tiles so an early 'v' tile doesn't
                # stall the DVE pipeline ahead of the PE-feeding tiles.
                def emit_v(t=t, w=w, xy=xy, m=m, ypl=ypl):
                    mxy = wp.tile([P, 2 * F * w], bf16, tag="mx")
                    nc.vector.tensor_tensor(
                        mxy[:].rearrange("p (k c) -> p k c", k=2 * F),
                        xy[:].rearrange("p (k c) -> p k c", k=2 * F),
                        m[:].unsqueeze(1).broadcast_to((P, 2 * F, w)),
                        op=Alu.mult)
                    mxv = mxy[:, 0:F * w]
                    myv = mxy[:, F * w:2 * F * w]
                    cw = sp.tile([P, F * w], bf16, tag="cw")
                    nc.vector.scalar_tensor_tensor(
                        cw[:], mxv, 1.0, ypl, op0=Alu.mult, op1=Alu.mult,
                        accum_out=acc[:, N_TILES + t:N_TILES + t + 1])
                    cw2 = sp.tile([P, F * w], bf16, tag="cw2")
                    nc.vector.scalar_tensor_tensor(
                        cw2[:], myv, 1.0, myv, op0=Alu.mult, op1=Alu.mult,
                        accum_out=acc[:, t:t + 1])
                    sq2 = sp.tile([P, F * w], bf16, tag="sq2")
                    nc.scalar.activation(sq2[:], mxv, Sq,
                                         accum_out=acc[:, 2 * N_TILES + t:
                                                       2 * N_TILES + t + 1])
                    sq3 = sp.tile([P, w], bf16, tag="sq3")
                    nc.scalar.activation(sq3[:], xy[:, 0:w], Sq, scale=sqa,
                                         accum_out=acc[:, 3 * N_TILES + t:
                                                       3 * N_TILES + t + 1])
                v_pending.append((t + V_DEFER, emit_v))

            while v_pending and v_pending[0][0] <= t:
                v_pending.pop(0)[1]()

        for _, fn in v_pending:
            fn()
        nc.sync.dma_start(o_d, acc[:])

    nc.compile()
    return nc


def _shard(a: np.ndarray) -> list[np.ndarray]:
    import ml_dtypes
    flat = a.reshape(-1)
    pad = np.zeros(PAD_CELLS * F, dtype=a.dtype)
    flat = np.concatenate([flat, pad]).astype(ml_dtypes.bfloat16)
    # (cores, P, cells, F) -> feature-plane layout (cores, P, F, cells)
    pc = flat.reshape(N_CORES, P, CPP, F).transpose(0, 1, 3, 2)
    pc = pc.reshape(N_CORES, P, FD)
    return [np.ascontiguousarray(pc[i]) for i in range(N_CORES)]


def kernel(x: np.ndarray, y: np.ndarray) -> np.ndarray:
    global _compiled
    if _compiled is None:
        _compiled = _build()
    nc = _compiled

    from concourse.bass_utils import run_bass_kernel_spmd

    xs = _shard(np.asarray(x, dtype=np.float32))
    ys = _shard(np.asarray(y, dtype=np.float32))
    in_maps = [{"x": xs[i], "y": ys[i]} for i in range(N_CORES)]
    res = run_bass_kernel_spmd(nc, in_maps, core_ids=list(range(N_CORES)))

    T = N_TILES
    vt = [t for t in range(T) if MYSQ[t] == 'v']
    total = np.float64(0.0)
    for r in res.results:
        o = r["o"].astype(np.float64)
        g = r["g"].astype(np.float64)
        trA = np.trace(g[:, 0:128])
        trB = np.trace(g[:, 128:256])
        trC = np.trace(g[:, 256:384])
        trD = np.trace(g[:, 384:512])
        myq = trD + sum(o[:, t].sum() for t in range(T) if MYSQ[t] != 'p')
        cross = trA + sum(o[:, T + t].sum() for t in vt)
        mxq = trB + sum(o[:, 2 * T + t].sum() for t in vt)
        bg = ALPHA * trC + sum(o[:, 3 * T + t].sum() for t in vt)
        total += myq - 2.0 * cross + (1.0 - ALPHA) * mxq + bg
    return np.float32(total)


# revision 31
# speedup vs baseline: 1.0269x; 1.0269x over previous
"""Masked-loss kernel for nn_MLoss_9715216024200 on 8 Trainium2 NeuronCores.

loss = sum(where(y[...,0]>0.5, (y-x)^2 - a*x^2, 0)) + a*sum(x[...,0]^2)
with x,y f32 (256, 10647, 5); output is a f32 scalar.

Sharding: flatten to cells, pad with 256 zero cells (neutral: y0=0 ->
mask 0, x=0 -> no bg term), split across (8 cores, 128 partitions,
2662 cells), and ship the shards as bf16 in FEATURE-PLANE layout: each
core's x (and y) is [128, 5*2662] with the 5 features stored as packed
per-feature planes.  bf16 halves the HBM stream to ~19us at the 360GB/s
DMA roofline (the loss tolerates it: rel err ~1e-4 << 2e-2), and the
plane layout makes the mask a PACKED [128, w] tensor -- no per-feature
replication, so the mask costs 0.28ns/cell (DVE tensor_scalar 4x mode)
instead of 1.4ns/elem on Pool.

Per-core math uses mask idempotence (m in {0,1} => m^2 = m):

  sum(m*(d^2 - a*x^2)) = sum(m*y^2) - 2*sum((m*x) o y) + (1-a)*sum((m*x)^2)

The big sums are Gram-matrix diagonals on the otherwise idle PE: per
128-cell block and feature plane, matmul(psA += mx_f^T y_f),
matmul(psB += mx_f^T x_f), matmul(psC += x0^T x0), and for 'p'-mode
tiles matmul(psD += m^T y2_f) (53ns per matmul; the host takes trace()
of the four staged 128x128 Grams).  PE matmuls are emitted PE_LAG tiles
behind the stream so PE ramps to full clock once and stays there.
sum(m*y^2) is per-tile routed by MYSQ: 'a' = my=m*y (part of one
10-plane masked multiply) + ACT Square-accum; 'p' = y2=y*y (DVE/ACT/
Pool by knob) + PE D-Gram; 'v' = off-PE (DVE scalar_tensor_tensor
accum for cross + my^2, ACT for mx^2 + bg -- NOT tensor_tensor_reduce,
which hard-crashes real TRN2 devices).  The single 'v' tile goes FIRST
in the stream (its serial chain runs early, deferred V_DEFER tiles so
it doesn't delay the PE pipeline), the o accumulator export issues
mid-stream, and the last tile streams x per plane with mx + Gram
matmuls pipelined per plane, so the psum close trails the final DMA by
well under a microsecond.  Each Gram gets its own PSUM bank (hardware
tracks accumulation groups per bank).  Host combines in f64 from the
o accumulator columns and the four staged Gram traces.

Per-elem rates: DVE tt/ts packed bf16 0.52/0.28, DVE stt 1.06, ACT
square 0.83 (+0.6us/instr), Pool tt 1.98, PE 0.41 per Gram term.
Engines land at ~10-16us each, near the ~19.3us DMA stream.
"""
import sys

for _p in ('/opt/trn_rl_repo',):
    if _p in sys.path:
        sys.path.remove(_p)
    sys.path.insert(0, _p)

import os as _os
import numpy as np

B, C, F = 256, 10647, 5
THRESH = 0.5
ALPHA = 0.1
N_CORES = 8
P = 128
CELLS = B * C                      # 2,725,632
CPP = 2662                         # cells/partition; 8*128*2662 = 2,725,888
PAD_CELLS = N_CORES * P * CPP - CELLS   # 256
FD = CPP * F                       # 13310 elems per partition per core

_ts = _os.environ.get('TILE_SIZES', '')
TILE_SIZES = ([int(v) for v in _ts.split(',')] if _ts
              else [102] + [256] * 10)
assert sum(TILE_SIZES) == CPP
N_TILES = len(TILE_SIZES)
# per-tile my^2 route: 'a' = my + ACT square-accum, 'p' = y2 + PE D-Gram,
# 'v' = off-PE tile (DVE ttr cross + my^2, ACT mx^2 + bg) -- put 'v'
# tiles FIRST in the stream so their slow chains run early
_mm = _os.environ.get('MYSQ', 'v,a,p,a,p,a,p,a,p,p,p')
MYSQ = _mm.split(',')
assert len(MYSQ) == N_TILES
# engine for y2 on 'p' tiles: tiles listed in Y2_ACT use ACT, Y2_POOL use
# Pool, the rest DVE
_ya = _os.environ.get('Y2_ACT', '8')
Y2_ACT = set(int(v) for v in _ya.split(',') if v != '')
_yp = _os.environ.get('Y2_POOL', '2,4,6')
Y2_POOL = set(int(v) for v in _yp.split(',') if v != '')
# 'a' tiles whose my-planes run on Pool (separate from the mx multiply)
_mg = _os.environ.get('MY_POOL', '1')
MY_POOL = set(int(v) for v in _mg.split(',') if v != '')
# flush tile t's PE matmuls after tile t+PE_LAG's products are emitted
PE_LAG = int(_os.environ.get('PE_LAG', '2'))
# defer 'v' tiles' compute emission by this many tiles
V_DEFER = int(_os.environ.get('V_DEFER', '2'))
BUFS = [int(v) for v in _os.environ.get('BUFS', '6,6,5,3').split(',')]

_compiled = None


def _build():
    from contextlib import ExitStack
    import concourse.tile as tile
    from concourse import bacc, mybir

    sqa = float(np.sqrt(ALPHA))

    nc = bacc.Bacc("TRN2", target_bir_lowering=False, debug=False,
                   enable_asserts=True, num_devices=N_CORES)
    bf16 = mybir.dt.bfloat16
    f32 = mybir.dt.float32
    x_d = nc.dram_tensor("x", [P, FD], bf16, kind="ExternalInput").ap()
    y_d = nc.dram_tensor("y", [P, FD], bf16, kind="ExternalInput").ap()
    o_d = nc.dram_tensor("o", [P, 4 * N_TILES], f32, kind="ExternalOutput").ap()
    g_d = nc.dram_tensor("g", [P, 512], bf16, kind="ExternalOutput").ap()

    Sq = mybir.ActivationFunctionType.Square
    Alu = mybir.AluOpType

    pe_tiles = [t for t in range(N_TILES) if MYSQ[t] != 'v']
    last_pe = pe_tiles[-1]
    last_p = max((t for t in pe_tiles if MYSQ[t] == 'p'), default=-1)
    # psum col ranges: A=cross(mx,y)  B=(mx)^2  C=bg x0^2  D=m o y^2
    first_pe = [True, True, True, True]

    x3 = x_d.rearrange("p (f c) -> p f c", f=F)
    y3 = y_d.rearrange("p (f c) -> p f c", f=F)

    with tile.TileContext(nc) as tc, ExitStack() as ctx:
        xyp = ctx.enter_context(tc.tile_pool(name="xy", bufs=BUFS[0]))
        mp_ = ctx.enter_context(tc.tile_pool(name="m", bufs=BUFS[1]))
        wp = ctx.enter_context(tc.tile_pool(name="w", bufs=BUFS[2]))
        sp = ctx.enter_context(tc.tile_pool(name="s", bufs=BUFS[3]))
        ap_ = ctx.enter_context(tc.tile_pool(name="acc", bufs=1))
        pp = ctx.enter_context(tc.psum_pool(name="ps", bufs=1))

        acc = ap_.tile([P, 4 * N_TILES], f32)
        nc.vector.memset(acc[:], 0.0)
        gst = ap_.tile([P, 512], bf16)
        # one PSUM bank per Gram: hardware tracks accumulation groups
        # per bank, so interleaved A/B/C/D accumulation needs 4 banks
        psA = pp.tile([P, 512], f32)
        psB = pp.tile([P, 512], f32)
        psC = pp.tile([P, 512], f32)
        psD = pp.tile([P, 512], f32)

        def emit_pe(t, w, xy, m, mxv, y2v):
            # per 128-cell block: A/B per plane; C plane-0; D (if y2v) per
            # plane against the shared mask stationary
            last_ab = (t == last_pe)
            nb = (w + 127) // 128
            for j in range(nb):
                lo = j * 128
                wb = min(128, w - lo)
                lab = last_ab and (j == nb - 1)
                for f in range(F):
                    mxf = mxv[:, f * w + lo: f * w + lo + wb]
                    yf = xy[:, (F + f) * w + lo: (F + f) * w + lo + wb]
                    xf = xy[:, f * w + lo: f * w + lo + wb]
                    nc.tensor.matmul(psA[0:wb, 0:wb], mxf, yf,
                                     start=first_pe[0],
                                     stop=lab and f == F - 1,
                                     skip_group_check=True)
                    first_pe[0] = False
                    nc.tensor.matmul(psB[0:wb, 0:wb], mxf, xf,
                                     start=first_pe[1],
                                     stop=lab and f == F - 1,
                                     skip_group_check=True)
                    first_pe[1] = False
                    if y2v is not None:
                        y2f = y2v[:, f * w + lo: f * w + lo + wb]
                        nc.tensor.matmul(psD[0:wb, 0:wb],
                                         m[:, lo:lo + wb], y2f,
                                         start=first_pe[3],
                                         stop=(t == last_p and j == nb - 1
                                               and f == F - 1),
                                         skip_group_check=True)
                        first_pe[3] = False
                x0b = xy[:, lo:lo + wb]
                nc.tensor.matmul(psC[0:wb, 0:wb], x0b, x0b,
                                 start=first_pe[2], stop=lab,
                                 skip_group_check=True)
                first_pe[2] = False
            if last_ab:
                # stage Grams to SBUF + export (overlaps the tail)
                nc.vector.tensor_copy(gst[:, 0:128], psA[:, 0:128])
                nc.scalar.copy(gst[:, 128:256], psB[:, 0:128])
                nc.vector.tensor_copy(gst[:, 256:384], psC[:, 0:128])
                nc.scalar.copy(gst[:, 384:512], psD[:, 0:128])
                nc.sync.dma_start(g_d, gst[:])

        pe_pending = []
        v_pending = []

        def emit_epilogue(t, w, xy, sl):
            while pe_pending:
                emit_pe(*pe_pending.pop(0))
            ypl = xy[:, F * w:2 * F * w]
            m = mp_.tile([P, w], bf16, tag="m")
            nc.vector.tensor_scalar(m[:], xy[:, F * w:F * w + w],
                                    THRESH, None, op0=Alu.is_gt)
            y2t = wp.tile([P, F * w], bf16, tag="y2")
            nc.vector.tensor_tensor(y2t[:], ypl, ypl, op=Alu.mult)
            mxt = wp.tile([P, F * w], bf16, tag="mx")
            nb = (w + 127) // 128
            for f in range(F):
                nc.sync.dma_start(
                    xy[:, f * w:(f + 1) * w].unsqueeze(1),
                    x3[:, f:f + 1, sl])
                nc.vector.tensor_tensor(mxt[:, f * w:(f + 1) * w],
                                        xy[:, f * w:(f + 1) * w], m[:],
                                        op=Alu.mult)
                for j in range(nb):
                    lo = j * 128
                    wb = min(128, w - lo)
                    fin = (f == F - 1 and j == nb - 1)
                    mxf = mxt[:, f * w + lo: f * w + lo + wb]
                    yf = xy[:, (F + f) * w + lo: (F + f) * w + lo + wb]
                    xf = xy[:, f * w + lo: f * w + lo + wb]
                    y2f = y2t[:, f * w + lo: f * w + lo + wb]
                    nc.tensor.matmul(psA[0:wb, 0:wb], mxf, yf,
                                     start=first_pe[0], stop=fin,
                                     skip_group_check=True)
                    first_pe[0] = False
                    nc.tensor.matmul(psB[0:wb, 0:wb], mxf, xf,
                                     start=first_pe[1], stop=fin,
                                     skip_group_check=True)
                    first_pe[1] = False
                    nc.tensor.matmul(psD[0:wb, 0:wb],
                                     m[:, lo:lo + wb], y2f,
                                     start=first_pe[3], stop=fin,
                                     skip_group_check=True)
                    first_pe[3] = False
                    if f == 0:
                        nc.tensor.matmul(psC[0:wb, 0:wb], xf, xf,
                                         start=first_pe[2],
                                         stop=(j == nb - 1),
                                         skip_group_check=True)
                        first_pe[2] = False
            nc.vector.tensor_copy(gst[:, 0:128], psA[:, 0:128])
            nc.scalar.copy(gst[:, 128:256], psB[:, 0:128])
            nc.vector.tensor_copy(gst[:, 256:384], psC[:, 0:128])
            nc.scalar.copy(gst[:, 384:512], psD[:, 0:128])
            nc.sync.dma_start(g_d, gst[:])

        off = 0
        for t, w in enumerate(TILE_SIZES):
            epilogue = (t == N_TILES - 1 and MYSQ[t] == 'p')
            xy = xyp.tile([P, 2 * F * w], bf16, tag="xy")
            sl = slice(off, off + w)
            off += w
            # y first (mask depends on it), then x; plane-strided DMAs
            nc.sync.dma_start(
                xy[:, F * w:2 * F * w].rearrange("p (f c) -> p f c", f=F),
                y3[:, :, sl])
            if epilogue:
                # final tile: stream x per plane, pipelining mx + Gram
                # matmuls behind each plane so the psum close trails the
                # last DMA by well under a microsecond
                emit_epilogue(t, w, xy, sl)
                continue
            nc.sync.dma_start(
                xy[:, 0:F * w].rearrange("p (f c) -> p f c", f=F),
                x3[:, :, sl])
            xpl = xy[:, 0:F * w]          # x planes
            ypl = xy[:, F * w:2 * F * w]  # y planes

            # packed per-cell mask (4x DVE tensor_scalar)
            m = mp_.tile([P, w], bf16, tag="m")
            nc.vector.tensor_scalar(m[:], xy[:, F * w:F * w + w],
                                    THRESH, None, op0=Alu.is_gt)

            mode = MYSQ[t]
            y2v = None
            if mode == 'p':
                mxt = wp.tile([P, F * w], bf16, tag="mx")
                nc.vector.tensor_tensor(
                    mxt[:].rearrange("p (f c) -> p f c", f=F),
                    xpl.rearrange("p (f c) -> p f c", f=F),
                    m[:].unsqueeze(1).broadcast_to((P, F, w)), op=Alu.mult)
                mxv = mxt[:]
                y2t = wp.tile([P, F * w], bf16, tag="y2")
                y2_eng = (nc.scalar if t in Y2_ACT
                          else nc.gpsimd if t in Y2_POOL else nc.vector)
                if t in Y2_ACT:
                    nc.scalar.activation(y2t[:], ypl, Sq)
                else:
                    y2_eng.tensor_tensor(y2t[:], ypl, ypl, op=Alu.mult)
                y2v = y2t[:]
            elif mode == 'v':
                mxv = myv = None
            else:
                # masked multiply of all 10 planes (or x/y split DVE/Pool)
                mxy = wp.tile([P, 2 * F * w], bf16, tag="mx")
                if t in MY_POOL and mode == 'a':
                    nc.vector.tensor_tensor(
                        mxy[:, 0:F * w].rearrange("p (f c) -> p f c", f=F),
                        xpl.rearrange("p (f c) -> p f c", f=F),
                        m[:].unsqueeze(1).broadcast_to((P, F, w)),
                        op=Alu.mult)
                    nc.gpsimd.tensor_tensor(
                        mxy[:, F * w:].rearrange("p (f c) -> p f c", f=F),
                        ypl.rearrange("p (f c) -> p f c", f=F),
                        m[:].unsqueeze(1).broadcast_to((P, F, w)),
                        op=Alu.mult)
                else:
                    nc.vector.tensor_tensor(
                        mxy[:].rearrange("p (k c) -> p k c", k=2 * F),
                        xy[:].rearrange("p (k c) -> p k c", k=2 * F),
                        m[:].unsqueeze(1).broadcast_to((P, 2 * F, w)),
                        op=Alu.mult)
                mxv = mxy[:, 0:F * w]
                myv = mxy[:, F * w:2 * F * w]
                if mode == 'a':
                    sq = sp.tile([P, F * w], bf16, tag="sq")
                    nc.scalar.activation(sq[:], myv, Sq,
                                         accum_out=acc[:, t:t + 1])

            if mode != 'v':
                pe_pending.append((t, w, xy, m, mxv, y2v))
                while pe_pending and (pe_pending[0][0] + PE_LAG <= t
                                      or t == last_pe):
                    emit_pe(*pe_pending.pop(0))
            else:
                # off-PE tile: cross + my^2 on DVE ttr, mx^2 + bg on ACT.
                # Deferred V_DEFER tiles so an early 'v' tile doesn't
                # stall the DVE pipeline ahead of the PE-feeding tiles.
                def emit_v(t=t, w=w, xy=xy, m=m, ypl=ypl):
                    mxy = wp.tile([P, 2 * F * w], bf16, tag="mx")
                    nc.vector.tensor_tensor(
                        mxy[:].rearrange("p (k c) -> p k c", k=2 * F),
                        xy[:].rearrange("p (k c) -> p k c", k=2 * F),
                        m[:].unsqueeze(1).broadcast_to((P, 2 * F, w)),
                        op=Alu.mult)
                    mxv = mxy[:, 0:F * w]
                    myv = mxy[:, F * w:2 * F * w]
                    cw = sp.tile([P, F * w], bf16, tag="cw")
                    nc.vector.scalar_tensor_tensor(
                        cw[:], mxv, 1.0, ypl, op0=Alu.mult, op1=Alu.mult,
                        accum_out=acc[:, N_TILES + t:N_TILES + t + 1])
                    cw2 = sp.tile([P, F * w], bf16, tag="cw2")
                    nc.vector.scalar_tensor_tensor(
                        cw2[:], myv, 1.0, myv, op0=Alu.mult, op1=Alu.mult,
                        accum_out=acc[:, t:t + 1])
                    sq2 = sp.tile([P, F * w], bf16, tag="sq2")
                    nc.scalar.activation(sq2[:], mxv, Sq,
                                         accum_out=acc[:, 2 * N_TILES + t:
                                                       2 * N_TILES + t + 1])
                    sq3 = sp.tile([P, w], bf16, tag="sq3")
                    nc.scalar.activation(sq3[:], xy[:, 0:w], Sq, scale=sqa,
                                         accum_out=acc[:, 3 * N_TILES + t:
                                                       3 * N_TILES + t + 1])
                v_pending.append((t + V_DEFER, emit_v))

            while v_pending and v_pending[0][0] <= t:
                v_pending.pop(0)[1]()

        for _, fn in v_pending:
            fn()
        nc.sync.dma_start(o_d, acc[:])

    nc.compile()
    return nc


def _shard(a: np.ndarray) -> list[np.ndarray]:
    import ml_dtypes
    flat = a.reshape(-1)
    pad = np.zeros(PAD_CELLS * F, dtype=a.dtype)
    flat = np.concatenate([flat, pad]).astype(ml_dtypes.bfloat16)
    # (cores, P, cells, F) -> feature-plane layout (cores, P, F, cells)
    pc = flat.reshape(N_CORES, P, CPP, F).transpose(0, 1, 3, 2)
    pc = pc.reshape(N_CORES, P, FD)
    return [np.ascontiguousarray(pc[i]) for i in range(N_CORES)]


def kernel(x: np.ndarray, y: np.ndarray) -> np.ndarray:
    global _compiled
    if _compiled is None:
        _compiled = _build()
    nc = _compiled

    from concourse.bass_utils import run_bass_kernel_spmd

    xs = _shard(np.asarray(x, dtype=np.float32))
    ys = _shard(np.asarray(y, dtype=np.float32))
    in_maps = [{"x": xs[i], "y": ys[i]} for i in range(N_CORES)]
    res = run_bass_kernel_spmd(nc, in_maps, core_ids=list(range(N_CORES)))

    T = N_TILES
    vt = [t for t in range(T) if MYSQ[t] == 'v']
    total = np.float64(0.0)
    for r in res.results:
        o = r["o"].astype(np.float64)
        g = r["g"].astype(np.float64)
        trA = np.trace(g[:, 0:128])
        trB = np.trace(g[:, 128:256])
        trC = np.trace(g[:, 256:384])
        trD = np.trace(g[:, 384:512])
        myq = trD + sum(o[:, t].sum() for t in range(T) if MYSQ[t] != 'p')
        cross = trA + sum(o[:, T + t].sum() for t in vt)
        mxq = trB + sum(o[:, 2 * T + t].sum() for t in vt)
        bg = ALPHA * trC + sum(o[:, 3 * T + t].sum() for t in vt)
        total += myq - 2.0 * cross + (1.0 - ALPHA) * mxq + bg
    return np.float32(total)


# revision 33
# speedup vs baseline: 1.0307x; 1.0037x over previous
"""Masked-loss kernel for nn_MLoss_9715216024200 on 8 Trainium2 NeuronCores.

loss = sum(where(y[...,0]>0.5, (y-x)^2 - a*x^2, 0)) + a*sum(x[...,0]^2)
with x,y f32 (256, 10647, 5); output is a f32 scalar.

Sharding: flatten to cells, pad with 256 zero cells (neutral: y0=0 ->
mask 0, x=0 -> no bg term), split across (8 cores, 128 partitions,
2662 cells), and ship the shards as bf16 in FEATURE-PLANE layout: each
core's x (and y) is [128, 5*2662] with the 5 features stored as packed
per-feature planes.  bf16 halves the HBM stream to ~19us at the 360GB/s
DMA roofline (the loss tolerates it: rel err ~1e-4 << 2e-2), and the
plane layout makes the mask a PACKED [128, w] tensor -- no per-feature
replication, so the mask costs 0.28ns/cell (DVE tensor_scalar 4x mode)
instead of 1.4ns/elem on Pool.

Per-core math uses mask idempotence (m in {0,1} => m^2 = m):

  sum(m*(d^2 - a*x^2)) = sum(m*y^2) - 2*sum((m*x) o y) + (1-a)*sum((m*x)^2)

The big sums are Gram-matrix diagonals on the otherwise idle PE: per
128-cell block and feature plane, matmul(psA += mx_f^T y_f),
matmul(psB += mx_f^T x_f), matmul(psC += x0^T x0), and for 'p'-mode
tiles matmul(psD += m^T y2_f) (53ns per matmul; the host takes trace()
of the four staged 128x128 Grams).  PE matmuls are emitted PE_LAG tiles
behind the stream so PE ramps to full clock once and stays there.
sum(m*y^2) is per-tile routed by MYSQ: 'a' = my=m*y (part of one
10-plane masked multiply) + ACT Square-accum; 'p' = y2=y*y (DVE/ACT/
Pool by knob) + PE D-Gram; 'v' = off-PE (DVE scalar_tensor_tensor
accum for cross + my^2, ACT for mx^2 + bg -- NOT tensor_tensor_reduce,
which hard-crashes real TRN2 devices).  The single 'v' tile goes FIRST
in the stream (its serial chain runs early, deferred V_DEFER tiles so
it doesn't delay the PE pipeline), the o accumulator export issues
mid-stream, and the last tile streams x per plane with mx + Gram
matmuls pipelined per plane, so the psum close trails the final DMA by
well under a microsecond.  Each Gram gets its own PSUM bank (hardware
tracks accumulation groups per bank).  Host combines in f64 from the
o accumulator columns and the four staged Gram traces.

Per-elem rates: DVE tt/ts packed bf16 0.52/0.28, DVE stt 1.06, ACT
square 0.83 (+0.6us/instr), Pool tt 1.98, PE 0.41 per Gram term.
Engines land at ~10-16us each, near the ~19.3us DMA stream.
"""
import sys

for _p in ('/opt/trn_rl_repo',):
    if _p in sys.path:
        sys.path.remove(_p)
    sys.path.insert(0, _p)

import os as _os
import numpy as np

B, C, F = 256, 10647, 5
THRESH = 0.5
ALPHA = 0.1
N_CORES = 8
P = 128
CELLS = B * C                      # 2,725,632
CPP = 2662                         # cells/partition; 8*128*2662 = 2,725,888
PAD_CELLS = N_CORES * P * CPP - CELLS   # 256
FD = CPP * F                       # 13310 elems per partition per core

_ts = _os.environ.get('TILE_SIZES', '')
TILE_SIZES = ([int(v) for v in _ts.split(',')] if _ts
              else [102] + [256] * 10)
assert sum(TILE_SIZES) == CPP
N_TILES = len(TILE_SIZES)
# per-tile my^2 route: 'a' = my + ACT square-accum, 'p' = y2 + PE D-Gram,
# 'v' = off-PE tile (DVE ttr cross + my^2, ACT mx^2 + bg) -- put 'v'
# tiles FIRST in the stream so their slow chains run early
_mm = _os.environ.get('MYSQ', 'v,a,p,a,p,a,p,a,p,p,p')
MYSQ = _mm.split(',')
assert len(MYSQ) == N_TILES
# engine for y2 on 'p' tiles: tiles listed in Y2_ACT use ACT, Y2_POOL use
# Pool, the rest DVE
_ya = _os.environ.get('Y2_ACT', '8')
Y2_ACT = set(int(v) for v in _ya.split(',') if v != '')
_yp = _os.environ.get('Y2_POOL', '2,4,6')
Y2_POOL = set(int(v) for v in _yp.split(',') if v != '')
# 'a' tiles whose my-planes run on Pool (separate from the mx multiply)
_mg = _os.environ.get('MY_POOL', '1')
MY_POOL = set(int(v) for v in _mg.split(',') if v != '')
# flush tile t's PE matmuls after tile t+PE_LAG's products are emitted
PE_LAG = int(_os.environ.get('PE_LAG', '2'))
# defer 'v' tiles' compute emission by this many tiles
V_DEFER = int(_os.environ.get('V_DEFER', '3'))
# dummy matmuls into a scratch psum right after tile 0's y lands, so the
# PE p-state ramps to full clock before the real Gram stream begins
PE_WARM = int(_os.environ.get('PE_WARM', '26'))
BUFS = [int(v) for v in _os.environ.get('BUFS', '6,6,5,3').split(',')]

_compiled = None


def _build():
    from contextlib import ExitStack
    import concourse.tile as tile
    from concourse import bacc, mybir

    sqa = float(np.sqrt(ALPHA))

    nc = bacc.Bacc("TRN2", target_bir_lowering=False, debug=False,
                   enable_asserts=True, num_devices=N_CORES)
    bf16 = mybir.dt.bfloat16
    f32 = mybir.dt.float32
    x_d = nc.dram_tensor("x", [P, FD], bf16, kind="ExternalInput").ap()
    y_d = nc.dram_tensor("y", [P, FD], bf16, kind="ExternalInput").ap()
    o_d = nc.dram_tensor("o", [P, 4 * N_TILES], f32, kind="ExternalOutput").ap()
    g_d = nc.dram_tensor("g", [P, 512], bf16, kind="ExternalOutput").ap()

    Sq = mybir.ActivationFunctionType.Square
    Alu = mybir.AluOpType

    pe_tiles = [t for t in range(N_TILES) if MYSQ[t] != 'v']
    last_pe = pe_tiles[-1]
    last_p = max((t for t in pe_tiles if MYSQ[t] == 'p'), default=-1)
    # psum col ranges: A=cross(mx,y)  B=(mx)^2  C=bg x0^2  D=m o y^2
    first_pe = [True, True, True, True]

    x3 = x_d.rearrange("p (f c) -> p f c", f=F)
    y3 = y_d.rearrange("p (f c) -> p f c", f=F)

    with tile.TileContext(nc) as tc, ExitStack() as ctx:
        xyp = ctx.enter_context(tc.tile_pool(name="xy", bufs=BUFS[0]))
        mp_ = ctx.enter_context(tc.tile_pool(name="m", bufs=BUFS[1]))
        wp = ctx.enter_context(tc.tile_pool(name="w", bufs=BUFS[2]))
        sp = ctx.enter_context(tc.tile_pool(name="s", bufs=BUFS[3]))
        ap_ = ctx.enter_context(tc.tile_pool(name="acc", bufs=1))
        pp = ctx.enter_context(tc.psum_pool(name="ps", bufs=1))

        acc = ap_.tile([P, 4 * N_TILES], f32)
        nc.vector.memset(acc[:], 0.0)
        gst = ap_.tile([P, 512], bf16)
        # one PSUM bank per Gram: hardware tracks accumulation groups
        # per bank, so interleaved A/B/C/D accumulation needs 4 banks
        psA = pp.tile([P, 512], f32)
        psB = pp.tile([P, 512], f32)
        psC = pp.tile([P, 512], f32)
        psD = pp.tile([P, 512], f32)
        if PE_WARM:
            psW = pp.tile([P, 512], f32)

        def emit_pe(t, w, xy, m, mxv, y2v):
            # per 128-cell block: A/B per plane; C plane-0; D (if y2v) per
            # plane against the shared mask stationary
            last_ab = (t == last_pe)
            nb = (w + 127) // 128
            for j in range(nb):
                lo = j * 128
                wb = min(128, w - lo)
                lab = last_ab and (j == nb - 1)
                for f in range(F):
                    mxf = mxv[:, f * w + lo: f * w + lo + wb]
                    yf = xy[:, (F + f) * w + lo: (F + f) * w + lo + wb]
                    xf = xy[:, f * w + lo: f * w + lo + wb]
                    nc.tensor.matmul(psA[0:wb, 0:wb], mxf, yf,
                                     start=first_pe[0],
                                     stop=lab and f == F - 1,
                                     skip_group_check=True)
                    first_pe[0] = False
                    nc.tensor.matmul(psB[0:wb, 0:wb], mxf, xf,
                                     start=first_pe[1],
                                     stop=lab and f == F - 1,
                                     skip_group_check=True)
                    first_pe[1] = False
                    if y2v is not None:
                        y2f = y2v[:, f * w + lo: f * w + lo + wb]
                        nc.tensor.matmul(psD[0:wb, 0:wb],
                                         m[:, lo:lo + wb], y2f,
                                         start=first_pe[3],
                                         stop=(t == last_p and j == nb - 1
                                               and f == F - 1),
                                         skip_group_check=True)
                        first_pe[3] = False
                x0b = xy[:, lo:lo + wb]
                nc.tensor.matmul(psC[0:wb, 0:wb], x0b, x0b,
                                 start=first_pe[2], stop=lab,
                                 skip_group_check=True)
                first_pe[2] = False
            if last_ab:
                # stage Grams to SBUF + export (overlaps the tail)
                nc.vector.tensor_copy(gst[:, 0:128], psA[:, 0:128])
                nc.scalar.copy(gst[:, 128:256], psB[:, 0:128])
                nc.vector.tensor_copy(gst[:, 256:384], psC[:, 0:128])
                nc.scalar.copy(gst[:, 384:512], psD[:, 0:128])
                nc.sync.dma_start(g_d, gst[:])

        pe_pending = []
        v_pending = []

        def emit_epilogue(t, w, xy, sl):
            while pe_pending:
                emit_pe(*pe_pending.pop(0))
            ypl = xy[:, F * w:2 * F * w]
            m = mp_.tile([P, w], bf16, tag="m")
            nc.vector.tensor_scalar(m[:], xy[:, F * w:F * w + w],
                                    THRESH, None, op0=Alu.is_gt)
            y2t = wp.tile([P, F * w], bf16, tag="y2")
            nc.vector.tensor_tensor(y2t[:], ypl, ypl, op=Alu.mult)
            mxt = wp.tile([P, F * w], bf16, tag="mx")
            nb = (w + 127) // 128
            for f in range(F):
                nc.sync.dma_start(
                    xy[:, f * w:(f + 1) * w].unsqueeze(1),
                    x3[:, f:f + 1, sl])
                nc.vector.tensor_tensor(mxt[:, f * w:(f + 1) * w],
                                        xy[:, f * w:(f + 1) * w], m[:],
                                        op=Alu.mult)
                for j in range(nb):
                    lo = j * 128
                    wb = min(128, w - lo)
                    fin = (f == F - 1 and j == nb - 1)
                    mxf = mxt[:, f * w + lo: f * w + lo + wb]
                    yf = xy[:, (F + f) * w + lo: (F + f) * w + lo + wb]
                    xf = xy[:, f * w + lo: f * w + lo + wb]
                    y2f = y2t[:, f * w + lo: f * w + lo + wb]
                    nc.tensor.matmul(psA[0:wb, 0:wb], mxf, yf,
                                     start=first_pe[0], stop=fin,
                                     skip_group_check=True)
                    first_pe[0] = False
                    nc.tensor.matmul(psB[0:wb, 0:wb], mxf, xf,
                                     start=first_pe[1], stop=fin,
                                     skip_group_check=True)
                    first_pe[1] = False
                    nc.tensor.matmul(psD[0:wb, 0:wb],
                                     m[:, lo:lo + wb], y2f,
                                     start=first_pe[3], stop=fin,
                                     skip_group_check=True)
                    first_pe[3] = False
                    if f == 0:
                        nc.tensor.matmul(psC[0:wb, 0:wb], xf, xf,
                                         start=first_pe[2],
                                         stop=(j == nb - 1),
                                         skip_group_check=True)
                        first_pe[2] = False
            nc.vector.tensor_copy(gst[:, 0:128], psA[:, 0:128])
            nc.scalar.copy(gst[:, 128:256], psB[:, 0:128])
            nc.vector.tensor_copy(gst[:, 256:384], psC[:, 0:128])
            nc.scalar.copy(gst[:, 384:512], psD[:, 0:128])
            nc.sync.dma_start(g_d, gst[:])

        off = 0
        for t, w in enumerate(TILE_SIZES):
            epilogue = (t == N_TILES - 1 and MYSQ[t] == 'p')
            xy = xyp.tile([P, 2 * F * w], bf16, tag="xy")
            sl = slice(off, off + w)
            off += w
            # y first (mask depends on it), then x; plane-strided DMAs
            nc.sync.dma_start(
                xy[:, F * w:2 * F * w].rearrange("p (f c) -> p f c", f=F),
                y3[:, :, sl])
            if epilogue:
                # final tile: stream x per plane, pipelining mx + Gram
                # matmuls behind each plane so the psum close trails the
                # last DMA by well under a microsecond
                emit_epilogue(t, w, xy, sl)
                continue
            nc.sync.dma_start(
                xy[:, 0:F * w].rearrange("p (f c) -> p f c", f=F),
                x3[:, :, sl])
            xpl = xy[:, 0:F * w]          # x planes
            ypl = xy[:, F * w:2 * F * w]  # y planes

            if t == 0 and PE_WARM:
                yw = xy[:, F * w:F * w + min(w, 128)]
                for i in range(PE_WARM):
                    nc.tensor.matmul(psW[0:min(w, 128), 0:min(w, 128)],
                                     yw, yw, start=(i == 0),
                                     stop=(i == PE_WARM - 1),
                                     skip_group_check=True)

            # packed per-cell mask (4x DVE tensor_scalar)
            m = mp_.tile([P, w], bf16, tag="m")
            nc.vector.tensor_scalar(m[:], xy[:, F * w:F * w + w],
                                    THRESH, None, op0=Alu.is_gt)

            mode = MYSQ[t]
            y2v = None
            if mode == 'p':
                mxt = wp.tile([P, F * w], bf16, tag="mx")
                nc.vector.tensor_tensor(
                    mxt[:].rearrange("p (f c) -> p f c", f=F),
                    xpl.rearrange("p (f c) -> p f c", f=F),
                    m[:].unsqueeze(1).broadcast_to((P, F, w)), op=Alu.mult)
                mxv = mxt[:]
                y2t = wp.tile([P, F * w], bf16, tag="y2")
                y2_eng = (nc.scalar if t in Y2_ACT
                          else nc.gpsimd if t in Y2_POOL else nc.vector)
                if t in Y2_ACT:
                    nc.scalar.activation(y2t[:], ypl, Sq)
                else:
                    y2_eng.tensor_tensor(y2t[:], ypl, ypl, op=Alu.mult)
                y2v = y2t[:]
            elif mode == 'v':
                mxv = myv = None
            else:
                # masked multiply of all 10 planes (or x/y split DVE/Pool)
                mxy = wp.tile([P, 2 * F * w], bf16, tag="mx")
                if t in MY_POOL and mode == 'a':
                    nc.vector.tensor_tensor(
                        mxy[:, 0:F * w].rearrange("p (f c) -> p f c", f=F),
                        xpl.rearrange("p (f c) -> p f c", f=F),
                        m[:].unsqueeze(1).broadcast_to((P, F, w)),
                        op=Alu.mult)
                    nc.gpsimd.tensor_tensor(
                        mxy[:, F * w:].rearrange("p (f c) -> p f c", f=F),
                        ypl.rearrange("p (f c) -> p f c", f=F),
                        m[:].unsqueeze(1).broadcast_to((P, F, w)),
                        op=Alu.mult)
                else:
                    nc.vector.tensor_tensor(
                        mxy[:].rearrange("p (k c) -> p k c", k=2 * F),
                        xy[:].rearrange("p (k c) -> p k c", k=2 * F),
                        m[:].unsqueeze(1).broadcast_to((P, 2 * F, w)),
                        op=Alu.mult)
                mxv = mxy[:, 0:F * w]
                myv = mxy[:, F * w:2 * F * w]
                if mode == 'a':
                    sq = sp.tile([P, F * w], bf16, tag="sq")
                    nc.scalar.activation(sq[:], myv, Sq,
                                         accum_out=acc[:, t:t + 1])

            if mode != 'v':
                pe_pending.append((t, w, xy, m, mxv, y2v))
                while pe_pending and (pe_pending[0][0] + PE_LAG <= t
                                      or t == last_pe):
                    emit_pe(*pe_pending.pop(0))
            else:
                # off-PE tile: cross + my^2 on DVE ttr, mx^2 + bg on ACT.
                # Deferred V_DEFER tiles so an early 'v' tile doesn't
                # stall the DVE pipeline ahead of the PE-feeding tiles.
                def emit_v(t=t, w=w, xy=xy, m=m, ypl=ypl):
                    mxy = wp.tile([P, 2 * F * w], bf16, tag="mx")
                    nc.vector.tensor_tensor(
                        mxy[:].rearrange("p (k c) -> p k c", k=2 * F),
                        xy[:].rearrange("p (k c) -> p k c", k=2 * F),
                        m[:].unsqueeze(1).broadcast_to((P, 2 * F, w)),
                        op=Alu.mult)
                    mxv = mxy[:, 0:F * w]
                    myv = mxy[:, F * w:2 * F * w]
                    cw = sp.tile([P, F * w], bf16, tag="cw")
                    nc.vector.scalar_tensor_tensor(
                        cw[:], mxv, 1.0, ypl, op0=Alu.mult, op1=Alu.mult,
                        accum_out=acc[:, N_TILES + t:N_TILES + t + 1])
                    cw2 = sp.tile([P, F * w], bf16, tag="cw2")
                    nc.vector.scalar_tensor_tensor(
                        cw2[:], myv, 1.0, myv, op0=Alu.mult, op1=Alu.mult,
                        accum_out=acc[:, t:t + 1])
                    sq2 = sp.tile([P, F * w], bf16, tag="sq2")
                    nc.scalar.activation(sq2[:], mxv, Sq,
                                         accum_out=acc[:, 2 * N_TILES + t:
                                                       2 * N_TILES + t + 1])
                    sq3 = sp.tile([P, w], bf16, tag="sq3")
                    nc.scalar.activation(sq3[:], xy[:, 0:w], Sq, scale=sqa,
                                         accum_out=acc[:, 3 * N_TILES + t:
                                                       3 * N_TILES + t + 1])
                v_pending.append((t + V_DEFER, emit_v))

            while v_pending and v_pending[0][0] <= t:
                v_pending.pop(0)[1]()

        for _, fn in v_pending:
            fn()
        nc.sync.dma_start(o_d, acc[:])

    nc.compile()
    return nc


def _shard(a: np.ndarray) -> list[np.ndarray]:
    import ml_dtypes
    flat = a.reshape(-1)
    pad = np.zeros(PAD_CELLS * F, dtype=a.dtype)
    flat = np.concatenate([flat, pad]).astype(ml_dtypes.bfloat16)
    # (cores, P, cells, F) -> feature-plane layout (cores, P, F, cells)
    pc = flat.reshape(N_CORES, P, CPP, F).transpose(0, 1, 3, 2)
    pc = pc.reshape(N_CORES, P, FD)
    return [np.ascontiguousarray(pc[i]) for i in range(N_CORES)]


def kernel(x: np.ndarray, y: np.ndarray) -> np.ndarray:
    global _compiled
    if _compiled is None:
        _compiled = _build()
    nc = _compiled

    from concourse.bass_utils import run_bass_kernel_spmd

    xs = _shard(np.asarray(x, dtype=np.float32))
    ys = _shard(np.asarray(y, dtype=np.float32))
    in_maps = [{"x": xs[i], "y": ys[i]} for i in range(N_CORES)]
    res = run_bass_kernel_spmd(nc, in_maps, core_ids=list(range(N_CORES)))

    T = N_TILES
    vt = [t for t in range(T) if MYSQ[t] == 'v']
    total = np.float64(0.0)
    for r in res.results:
        o = r["o"].astype(np.float64)
        g = r["g"].astype(np.float64)
        trA = np.trace(g[:, 0:128])
        trB = np.trace(g[:, 128:256])
        trC = np.trace(g[:, 256:384])
        trD = np.trace(g[:, 384:512])
        myq = trD + sum(o[:, t].sum() for t in range(T) if MYSQ[t] != 'p')
        cross = trA + sum(o[:, T + t].sum() for t in vt)
        mxq = trB + sum(o[:, 2 * T + t].sum() for t in vt)
        bg = ALPHA * trC + sum(o[:, 3 * T + t].sum() for t in vt)
        total += myq - 2.0 * cross + (1.0 - ALPHA) * mxq + bg
    return np.float32(total)
